# revision 1
# baseline (speedup 1.0000x reference)
"""Self-contained AGNN kernel for trn2 x8 NeuronCores.

kernel(**inputs) takes the FULL unsharded inputs (x, edge_index, W, b, betas)
and returns the FULL [N, 64] float32 output. Internally: nodes are sharded
over 8 cores (snake by degree), edges grouped by destination into padded
[128-node x K] grids, per-edge source rows fetched with dma_gather (4 SWDGE
queues, one per source-window chunk) from a replicated bf16 feature table,
cosine-attention softmax-aggregation on the vector engine, and the table
rebuilt + all-gathered between layers. Layer-0 table is precomputed on host;
pad slots are corrected analytically via a per-node pad-count tile instead of
a bias column; the residual x@W.T+b runs up-front on the tensor engine and is
fused into the last layer's batch loop.
"""

import sys
sys.path.insert(0, '/opt/trn_rl_repo')
import numpy as np
import ml_dtypes
from concourse import bass, bacc, tile, library_config
import concourse.mybir as mybir
from concourse.bass_utils import run_bass_kernel_spmd

F32 = mybir.dt.float32
BF16 = mybir.dt.bfloat16
I16 = mybir.dt.int16
ALU = mybir.AluOpType
ACTF = mybir.ActivationFunctionType

P = 128          # partitions
D = 64           # feature dim
EW = 128         # table row elems (bf16) = 256B
NCORES = 8
NCHUNK = 4       # src windows (= core-pairs)


# ---------------------------------------------------------------- host side

class Plan:
    pass


def preprocess(x, edge_index, B=8, CAP=40):
    """Build per-core arrays + compile-time schedule.

    Node placement: rank nodes by degree, snake over cores (edge balance).
    Within a core, sort nodes by max-chunk-degree so groups of 128 have a
    tight shared K. Table row of node = core*SLOTS + p*G + g.
    """
    N = x.shape[0]
    assert N % NCORES == 0
    shard = N // NCORES
    G = (shard + P - 1) // P
    SLOTS = G * P
    NTAB = NCORES * SLOTS
    WIN = NTAB // NCHUNK           # rows per src window (= 2 cores)
    assert WIN <= 32767 and SLOTS * 2 == WIN

    # self-loops are NOT gathered: their softmax contribution (cos=1 with
    # itself) is added locally on-device as exp(beta) * h_v
    src = edge_index[0].astype(np.int64)
    dst = edge_index[1].astype(np.int64)
    deg = np.bincount(dst, minlength=N)

    # assign nodes to core-PAIRS (= gather source windows) with a greedy that
    # spreads each destination's in-edges evenly over the 4 windows: the
    # per-batch slot count K is max over windows of per-window in-degree, so
    # even spread cuts grid padding from ~1.58x toward ~ceil(deg/4)/deg*4
    so = np.argsort(src, kind="stable")
    ds_by_src = dst[so]
    cnt_out = np.bincount(src, minlength=N)
    indptr = np.r_[0, np.cumsum(cnt_out)]
    gorder = np.argsort(-cnt_out, kind="stable")
    cnt = np.zeros((NCHUNK, N), dtype=np.int32)
    sizes = np.zeros(NCHUNK, dtype=np.int64)
    capn = N // NCHUNK
    pair_of = np.empty(N, dtype=np.int64)
    for v in gorder:
        dv = ds_by_src[indptr[v]:indptr[v + 1]]
        if len(dv):
            sc = cnt[:, dv].sum(axis=1).astype(np.float64)
        else:
            sc = np.zeros(NCHUNK)
        sc[sizes >= capn] = np.inf
        p = int(np.argmin(sc))
        pair_of[v] = p
        sizes[p] += 1
        if len(dv):
            np.add.at(cnt, (p, dv), 1)
    # within each pair: snake the 2 cores by in-degree for dst-work balance
    core_of = np.empty(N, dtype=np.int64)
    for pr in range(NCHUNK):
        nodes = np.where(pair_of == pr)[0]
        o2 = nodes[np.argsort(-deg[nodes], kind="stable")]
        pos2 = np.arange(len(o2)) % 2
        blk2 = np.arange(len(o2)) // 2
        cr = np.where(blk2 % 2 == 0, pos2, 1 - pos2)
        core_of[o2] = pr * 2 + cr

    chunk = core_of[src] >> 1                       # edge chunk = src core-pair
    degc = np.zeros((NCHUNK, N), dtype=np.int64)    # per-chunk in-degree
    for c in range(NCHUNK):
        degc[c] = np.bincount(dst[chunk == c], minlength=N)
    maxstat = degc.max(axis=0)

    # within-core ordering: by maxstat desc (dummies land last)
    p_of = np.empty(N, dtype=np.int64)
    g_of = np.empty(N, dtype=np.int64)
    n_real = np.zeros(NCORES, dtype=np.int64)
    kmax_pg = np.zeros((NCORES, G), dtype=np.int64)  # per (core, group) max stat
    core_nodes = []
    for c in range(NCORES):
        nodes = np.where(core_of == c)[0]
        n_real[c] = len(nodes)
        assert len(nodes) < SLOTS, "need >=1 dummy slot per core"
        o = nodes[np.argsort(-maxstat[nodes], kind="stable")]
        s = np.arange(len(o))
        g_of[o] = s // P
        p_of[o] = s % P
        core_nodes.append(o)
        gm = np.zeros(G, dtype=np.int64)
        np.maximum.at(gm, s // P, maxstat[o])
        kmax_pg[c] = gm

    row_of = core_of * SLOTS + p_of * G + g_of       # table row per real node

    # batch schedule (shared across cores): groups [g0, g0+Bb), K = max over
    # cores; pack groups while Bb*K stays under CAP (bounds tile sizes)
    def kfor(g0, g1):
        K = int(kmax_pg[:, g0:g1].max())
        return max(2, K)
    batches = []
    g0 = 0
    while g0 < G:
        Bb = 1
        while (g0 + Bb < G and Bb < B
               and (Bb + 1) * kfor(g0, g0 + Bb + 1) <= CAP):
            Bb += 1
        batches.append((g0, Bb, kfor(g0, g0 + Bb)))
        g0 += Bb

    # per-(batch,chunk) idx columns layout in the "idx" param
    col_off = []
    tot = 0
    for (g0, Bb, K) in batches:
        offs = []
        for c in range(NCHUNK):
            offs.append(tot)
            tot += 8 * Bb * K                        # cols = num_idxs/16
        col_off.append(offs)
    S_TOTAL = tot

    # dummy (pad) relative row per window: first dummy slot of core 2c
    pad_rel = np.empty(NCHUNK, dtype=np.int64)
    for c in range(NCHUNK):
        cc = 2 * c
        s = n_real[cc]                               # first dummy rank
        pad_rel[c] = (s % P) * G + s // P

    # slot k of each edge within its (dst, chunk) run
    key = dst * NCHUNK + chunk
    o = np.argsort(key, kind="stable")
    ks = np.empty(len(o), dtype=np.int64)
    sk = key[o]
    run_start = np.r_[0, np.flatnonzero(np.diff(sk)) + 1]
    run_id = np.zeros(len(o), dtype=np.int64)
    run_id[run_start[1:]] = 1
    run_id = np.cumsum(run_id)
    ks[o] = np.arange(len(o)) - run_start[run_id]

    # batch id / K per group
    batch_of_g = np.empty(G, dtype=np.int64)
    K_of_g = np.empty(G, dtype=np.int64)
    for bi, (g0, Bb, K) in enumerate(batches):
        batch_of_g[g0:g0 + Bb] = bi
        K_of_g[g0:g0 + Bb] = K

    # flat idx arrays per core, filled with per-window pad rows
    idx_arr = np.empty((NCORES, S_TOTAL * 16), dtype=np.int16)
    for bi, (g0, Bb, K) in enumerate(batches):
        for c in range(NCHUNK):
            lo = col_off[bi][c] * 16
            idx_arr[:, lo:lo + 128 * Bb * K] = pad_rel[c]

    # scatter real edges
    ec = core_of[dst]
    eg = g_of[dst]
    ep = p_of[dst]
    ebi = batch_of_g[eg]
    eK = K_of_g[eg]
    g0_of_b = np.array([b[0] for b in batches], dtype=np.int64)
    off_bc = np.array(col_off, dtype=np.int64)       # [nb, 4]
    base = off_bc[ebi, chunk] * 16
    pos = base + (((eg - g0_of_b[ebi]) * eK + ks) * P + ep)
    val = (row_of[src] - chunk * WIN).astype(np.int16)
    assert (ks < eK).all()
    idx_arr[ec, pos] = val

    # -> [core, 128, S_TOTAL] wrapped 16 + replicated
    idx_parts = np.empty((NCORES, P, S_TOTAL), dtype=np.int16)
    for cc in range(NCORES):
        w = idx_arr[cc].reshape(S_TOTAL, 16).T       # [16, S]
        idx_parts[cc] = np.tile(w, (8, 1))

    # per-core node-ordered inputs
    x_nm = np.zeros((NCORES, P, G * D), dtype=np.float32)
    xn_nm = np.zeros((NCORES, P, G * D), dtype=ml_dtypes.bfloat16)
    xtg = np.zeros((NCORES, D, G * P), dtype=ml_dtypes.bfloat16)
    t0 = np.zeros((NTAB, EW), dtype=ml_dtypes.bfloat16)   # replicated layer-0 table
    degg = np.zeros((NCORES, P, G), dtype=np.float32)     # total degree per slot

    xf = x.astype(np.float32)
    norm = np.maximum(np.sqrt((xf * xf).sum(1)), 1e-12)
    xn_full = (xf / norm[:, None])

    for cc in range(NCORES):
        o = core_nodes[cc]
        pp, gg = p_of[o], g_of[o]
        xo = xf[o]
        flat = np.zeros((P, G, D), dtype=np.float32)
        flat[pp, gg] = xo
        x_nm[cc] = flat.reshape(P, G * D)
        flatn = np.zeros((P, G, D), dtype=np.float32)
        flatn[pp, gg] = xn_full[o]
        xn_nm[cc] = flatn.reshape(P, G * D).astype(ml_dtypes.bfloat16)
        xt = np.zeros((G, P, D), dtype=np.float32)
        xt[gg, pp] = xo
        xtg[cc] = xt.reshape(G * P, D).T.astype(ml_dtypes.bfloat16)
        dg = np.zeros((P, G), dtype=np.float32)
        dg[pp, gg] = deg[o]
        degg[cc] = dg
        # layer-0 table rows for this core (row = p*G+g): xn + norm col
        trows = np.zeros((P, G, EW), dtype=np.float32)
        trows[pp, gg, 0:D] = xn_full[o]
        nrm = np.full((P, G), 1e-12, dtype=np.float32)
        nrm[pp, gg] = norm[o]
        trows[:, :, D] = nrm
        t0[cc * SLOTS:(cc + 1) * SLOTS] = trows.reshape(SLOTS, EW).astype(ml_dtypes.bfloat16)

    # pad-count correction per (p, g): NCHUNK*K_b(g) - deg  (subtract from
    # the raw denominator; e^beta self-loop term added in run())
    padc = np.zeros((NCORES, P, G), dtype=np.float32)
    for cc in range(NCORES):
        padc[cc] = NCHUNK * K_of_g[None, :].astype(np.float32) - degg[cc]

    pl = Plan()
    pl.N, pl.G, pl.SLOTS, pl.NTAB, pl.WIN, pl.S_TOTAL = N, G, SLOTS, NTAB, WIN, S_TOTAL
    pl.batches, pl.col_off = batches, col_off
    pl.idx = idx_parts
    pl.x_nm, pl.xn_nm, pl.xtg, pl.t0 = x_nm, xn_nm, xtg, t0
    pl.padc = padc
    pl.row_of = row_of
    pl.core_nodes = core_nodes
    pl.p_of, pl.g_of = p_of, g_of
    padded = sum(128 * Bb * K * NCHUNK for (g0, Bb, K) in batches) * NCORES
    pl.pad_factor = padded / len(src)
    return pl


# ---------------------------------------------------------------- device side

def build_kernel(pl, betas, debug=False, GB=4, DOTRED=True, NUMRED=False):
    G, WIN, NTAB, SLOTS = pl.G, pl.WIN, pl.NTAB, pl.SLOTS
    L = len(betas)
    nc = bacc.Bacc("TRN2", target_bir_lowering=False, debug=debug,
                   num_devices=NCORES, num_swdge_queues=4)

    x_nm = nc.dram_tensor("x_nm", [P, G * D], F32, kind="ExternalInput")
    xn_nm = nc.dram_tensor("xn_nm", [P, G * D], BF16, kind="ExternalInput")
    table0 = nc.dram_tensor("table0", [NTAB, EW], BF16, kind="ExternalInput")
    xtg = nc.dram_tensor("xtg", [D, G * P], BF16, kind="ExternalInput")
    wt = nc.dram_tensor("wt", [D, D], BF16, kind="ExternalInput")
    b_bc = nc.dram_tensor("b_bc", [P, D], F32, kind="ExternalInput")
    idx = nc.dram_tensor("idx", [P, pl.S_TOTAL], I16, kind="ExternalInput")
    pcl = nc.dram_tensor("pcl", [P, L * G], F32, kind="ExternalInput")
    out = nc.dram_tensor("out", [SLOTS, D], F32, kind="ExternalOutput")

    with tile.TileContext(nc) as tc:
        nc.gpsimd.load_library(library_config.mlp)
        with tc.tile_pool(name="persist", bufs=1) as wp, \
             tc.tile_pool(name="gat", bufs=GB) as gp, \
             tc.tile_pool(name="idxp", bufs=2) as ip, \
             tc.tile_pool(name="scr", bufs=1) as sp, \
             tc.tile_pool(name="town", bufs=2) as tp, \
             tc.tile_pool(name="fin", bufs=2) as fp_, \
             tc.tile_pool(name="psum", bufs=2, space="PSUM") as pp, \
             tc.tile_pool(name="dram", bufs=1, space="DRAM") as dp:

            H = wp.tile([P, G * D], F32)
            XN = wp.tile([P, G * D], BF16)
            INV = wp.tile([P, G], F32)
            NORMB = wp.tile([P, G], BF16)
            PC = wp.tile([P, L * G], F32)
            t_own = dp.tile([SLOTS, EW], BF16)
            rdram = dp.tile([P, G * D], F32)
            tables = [dp.tile([NTAB, EW], BF16, addr_space="Shared",
                              name=f"table{i}", tag=f"table{i}")
                      for i in range(1, L)]

            nc.sync.dma_start(out=H[:], in_=x_nm.ap())
            nc.sync.dma_start(out=XN[:], in_=xn_nm.ap())
            nc.sync.dma_start(out=PC[:], in_=pcl.ap())

            def h3():
                return H[:].rearrange("p (g e) -> p g e", g=G)

            def xn3():
                return XN[:].rearrange("p (g e) -> p g e", g=G)

            # ---- residual x @ W.T + b, computed up front into rdram
            WT = wp.tile([D, D], BF16)
            nc.sync.dma_start(out=WT[:], in_=wt.ap())
            BBC = wp.tile([P, D], F32)
            nc.sync.dma_start(out=BBC[:], in_=b_bc.ap())
            SLAB = 8
            for s0 in range(0, G, SLAB):
                sb = min(SLAB, G - s0)
                XT = tp.tile([D, SLAB * P], BF16, tag="xt")
                nc.sync.dma_start(out=XT[:, :sb * P], in_=xtg.ap()[:, s0 * P:(s0 + sb) * P])
                PS = pp.tile([P, SLAB * D], F32, space="PSUM", tag="ps")
                for j in range(sb):
                    nc.tensor.matmul(
                        out=PS[:, j * D:(j + 1) * D],
                        lhsT=XT[:, j * P:(j + 1) * P], rhs=WT[:],
                        start=True, stop=True)
                RS = tp.tile([P, SLAB * D], F32, tag="rs")
                bb = BBC[:].unsqueeze(1).broadcast_to([P, sb, D])
                nc.vector.tensor_tensor(
                    out=RS[:, :sb * D].rearrange("p (g e) -> p g e", g=sb),
                    in0=PS[:, :sb * D].rearrange("p (g e) -> p g e", g=sb),
                    in1=bb, op=ALU.add)
                nc.sync.dma_start(out=rdram[:, s0 * D:(s0 + sb) * D], in_=RS[:, :sb * D])

            def build_table(li):
                # norms of H rows -> INV, XN (normalized, bf16); write xn+norm
                # columns of t_own via strided DMAs; allgather -> table
                table = tables[li - 1]
                SQ = sp.tile([P, G * D], F32, tag="pr")
                nc.vector.tensor_tensor(out=SQ[:], in0=H[:], in1=H[:], op=ALU.mult)
                NSQ = sp.tile([P, G], F32, tag="nsq")
                nc.vector.tensor_reduce(
                    out=NSQ[:], in_=SQ[:].rearrange("p (g e) -> p g e", g=G),
                    axis=mybir.AxisListType.X, op=ALU.add)
                NRM = sp.tile([P, G], F32, tag="nrm")
                nc.scalar.activation(out=NRM[:], in_=NSQ[:], func=ACTF.Sqrt)
                nc.vector.tensor_scalar_max(out=NRM[:], in0=NRM[:], scalar1=1e-12)
                nc.vector.reciprocal(out=INV[:], in_=NRM[:])
                nc.vector.tensor_copy(out=NORMB[:], in_=NRM[:])
                inv3 = INV[:].unsqueeze(2).broadcast_to([P, G, D])
                nc.vector.tensor_tensor(out=xn3(), in0=h3(), in1=inv3, op=ALU.mult)
                tv = t_own[:].rearrange("(p g) e -> p g e", p=P)
                SLAB = 32   # keep DMA descriptor count per instruction < 8192
                for s0 in range(0, G, SLAB):
                    sb = min(SLAB, G - s0)
                    nc.sync.dma_start(
                        out=tv[:, s0:s0 + sb, 0:D],
                        in_=xn3()[:, s0:s0 + sb, :])
                    nc.sync.dma_start(
                        out=tv[:, s0:s0 + sb, D:D + 1],
                        in_=NORMB[:, s0:s0 + sb].unsqueeze(2))
                nc.gpsimd.collective_compute(
                    "AllGather", ALU.bypass,
                    replica_groups=[list(range(NCORES))],
                    ins=[t_own[:].opt()],
                    outs=[table[:].opt()])

            for li in range(L):
                beta = float(betas[li])
                ebeta = float(np.exp(beta))
                table_ap = table0.ap() if li == 0 else tables[li - 1][:]
                last = li == L - 1
                for bi, (g0, Bb, K) in enumerate(pl.batches):
                    W4 = NCHUNK * Bb * K              # slots per partition
                    cols = 8 * Bb * K                 # idx cols per chunk
                    IX = ip.tile([P, NCHUNK * cols], I16, tag="ix")
                    nc.sync.dma_start(
                        out=IX[:],
                        in_=idx.ap()[:, pl.col_off[bi][0]:pl.col_off[bi][0] + NCHUNK * cols])
                    GT = gp.tile([P, W4 * EW], BF16, tag="gt")
                    for c in range(NCHUNK):
                        slab = GT[:, c * Bb * K * EW:(c + 1) * Bb * K * EW]
                        nc.gpsimd.dma_gather(
                            slab.rearrange("p (j e) -> p j e", e=EW),
                            table_ap[c * WIN:(c + 1) * WIN, :],
                            IX[:, c * cols:(c + 1) * cols],
                            128 * Bb * K, 128 * Bb * K, EW, elem_step=EW,
                            single_packet=False, queue_num=c)
                    gt4 = GT[:].rearrange("p (j e) -> p j e", e=EW)

                    # dot products: per chunk multiply with xn broadcast
                    PR = sp.tile([P, W4 * D], BF16, tag="pr")
                    pr4c = PR[:].rearrange("p (cb k e) -> p cb k e", k=K, e=D)
                    xb = (xn3()[:, g0:g0 + Bb, :].unsqueeze(2)
                          .broadcast_to([P, Bb, K, D]))
                    for c in range(NCHUNK):
                        nc.vector.tensor_tensor(
                            out=pr4c[:, c * Bb:(c + 1) * Bb, :, :],
                            in0=gt4[:, c * Bb * K:(c + 1) * Bb * K, 0:D]
                                .rearrange("p (b k) e -> p b k e", k=K),
                            in1=xb, op=ALU.mult)

                    DOT = sp.tile([P, W4], F32, tag="dot")
                    if DOTRED:
                        nc.vector.tensor_reduce(
                            out=DOT[:],
                            in_=PR[:].rearrange("p (j e) -> p j e", e=D),
                            axis=mybir.AxisListType.X, op=ALU.add)
                    else:
                        TR = sp.tile([P, W4 * 32], BF16, tag="tr")
                        tr4 = TR[:].rearrange("p (j e) -> p j e", e=32)
                        prj = PR[:].rearrange("p (j e) -> p j e", e=D)
                        nc.vector.tensor_tensor(out=tr4[:, :, 0:32], in0=prj[:, :, 0:32],
                                                in1=prj[:, :, 32:64], op=ALU.add)
                        w = 16
                        while w >= 2:
                            nc.vector.tensor_tensor(
                                out=tr4[:, :, 0:w], in0=tr4[:, :, 0:w],
                                in1=tr4[:, :, w:2 * w], op=ALU.add)
                            w //= 2
                        nc.vector.tensor_reduce(out=DOT[:], in_=tr4[:, :, 0:2],
                                                axis=mybir.AxisListType.X, op=ALU.add)

                    EX = sp.tile([P, W4], BF16, tag="ex")
                    nc.scalar.activation(out=EX[:], in_=DOT[:],
                                         func=ACTF.Exp, scale=beta)
                    # denominator: sum + pad-count/self-loop correction
                    DEN = sp.tile([P, Bb], F32, tag="den")
                    exv = EX[:].rearrange("p (c b k) -> p b c k", c=NCHUNK, k=K)
                    nc.vector.tensor_reduce(
                        out=DEN[:], in_=exv,
                        axis=mybir.AxisListType.XY, op=ALU.add)
                    nc.vector.tensor_tensor(
                        out=DEN[:], in0=DEN[:],
                        in1=PC[:, li * G + g0:li * G + g0 + Bb], op=ALU.subtract)
                    R = sp.tile([P, Bb], F32, tag="rcp")
                    nc.vector.reciprocal(out=R[:], in_=DEN[:])
                    # weight including src norm (strided col D of the gather)
                    EXN = sp.tile([P, W4], BF16, tag="exn")
                    nc.vector.tensor_tensor(out=EXN[:], in0=EX[:],
                                            in1=gt4[:, :, D], op=ALU.mult)
                    # numer: PR reuse = GT * exn-broadcast, reduce over (c,k)
                    exb = EXN[:].unsqueeze(2).broadcast_to([P, W4, D])
                    pr4 = PR[:].rearrange("p (cb k e) -> p cb k e", k=K, e=D)
                    nc.vector.tensor_tensor(
                        out=pr4.rearrange("p cb k e -> p (cb k) e"),
                        in0=gt4[:, :, 0:D], in1=exb, op=ALU.mult)
                    NUM = sp.tile([P, Bb * D], F32, tag="num")
                    n3 = NUM[:].rearrange("p (b e) -> p b e", b=Bb)
                    if NUMRED:
                        nc.vector.tensor_reduce(
                            out=n3,
                            in_=PR[:].rearrange("p (c b k e) -> p b e c k",
                                                c=NCHUNK, k=K, e=D),
                            axis=mybir.AxisListType.XY, op=ALU.add)
                    else:
                        w = K
                        while w > 1:
                            if w % 2 == 1:
                                nc.vector.tensor_tensor(
                                    out=pr4[:, :, 0, :], in0=pr4[:, :, 0, :],
                                    in1=pr4[:, :, w - 1, :], op=ALU.add)
                                w -= 1
                            else:
                                nc.vector.tensor_tensor(
                                    out=pr4[:, :, 0:w // 2, :], in0=pr4[:, :, 0:w // 2, :],
                                    in1=pr4[:, :, w // 2:w, :], op=ALU.add)
                                w //= 2
                        nc.vector.tensor_tensor(out=n3, in0=pr4[:, 0:Bb, 0, :],
                                                in1=pr4[:, Bb:2 * Bb, 0, :], op=ALU.add)
                        nc.vector.tensor_tensor(out=n3, in0=n3,
                                                in1=pr4[:, 2 * Bb:3 * Bb, 0, :], op=ALU.add)
                        nc.vector.tensor_tensor(out=n3, in0=n3,
                                                in1=pr4[:, 3 * Bb:4 * Bb, 0, :], op=ALU.add)
                    # self-loop numerator term + normalize
                    nc.vector.scalar_tensor_tensor(
                        out=n3, in0=h3()[:, g0:g0 + Bb, :],
                        scalar=ebeta, in1=n3,
                        op0=ALU.mult, op1=ALU.add)
                    rb = R[:].unsqueeze(2).broadcast_to([P, Bb, D])
                    if not last:
                        nc.vector.tensor_tensor(
                            out=h3()[:, g0:g0 + Bb, :], in0=n3, in1=rb, op=ALU.mult)
                    else:
                        # fuse: out = relu(h_conv + res)
                        RSL = fp_.tile([P, CAPB * D], F32, tag="rsl")
                        nc.sync.dma_start(out=RSL[:, :Bb * D],
                                          in_=rdram[:, g0 * D:(g0 + Bb) * D])
                        FIN = fp_.tile([P, CAPB * D], F32, tag="fin")
                        f3 = FIN[:, :Bb * D].rearrange("p (b e) -> p b e", b=Bb)
                        nc.vector.tensor_tensor(out=f3, in0=n3, in1=rb, op=ALU.mult)
                        nc.vector.tensor_tensor(
                            out=f3, in0=f3,
                            in1=RSL[:, :Bb * D].rearrange("p (b e) -> p b e", b=Bb),
                            op=ALU.add)
                        nc.scalar.activation(out=FIN[:, :Bb * D], in_=FIN[:, :Bb * D],
                                             func=ACTF.Relu)
                        nc.sync.dma_start(
                            out=out.ap().rearrange("(p g) e -> p (g e)", p=P)
                            [:, g0 * D:(g0 + Bb) * D],
                            in_=FIN[:, :Bb * D])
                if li == 0:
                    nc.scalar.activation(out=H[:], in_=H[:], func=ACTF.Relu)
                if li < L - 1:
                    build_table(li + 1)
    nc.compile()
    return nc


CAPB = 8  # max Bb for the fin tile size


# ---------------------------------------------------------------- runner

_CACHE = {}


def run(x, edge_index, W, b, betas, B=8, CAP=24, GB=4, DOTRED=True, NUMRED=False,
        trace=False):
    global CAPB
    CAPB = B
    key = (x.shape, edge_index.shape, B, CAP, GB, DOTRED, NUMRED)
    pl = preprocess(np.asarray(x), np.asarray(edge_index), B=B, CAP=CAP)
    if key in _CACHE:
        nc = _CACHE[key]
    else:
        nc = build_kernel(pl, np.asarray(betas), GB=GB, DOTRED=DOTRED, NUMRED=NUMRED)
        _CACHE[key] = nc

    betas_np = np.asarray(betas, dtype=np.float32)
    L = len(betas_np)
    G = pl.G
    wt_np = np.ascontiguousarray(np.asarray(W).T).astype(ml_dtypes.bfloat16)
    bbc_np = np.tile(np.asarray(b, dtype=np.float32)[None, :], (P, 1))
    in_maps = []
    for c in range(NCORES):
        # pcl[l] = padcount - e^beta_l  (device does DEN = sum - pcl)
        pcl = np.empty((P, L * G), dtype=np.float32)
        for l in range(L):
            pcl[:, l * G:(l + 1) * G] = pl.padc[c] - np.exp(betas_np[l])
        in_maps.append({
            "x_nm": pl.x_nm[c],
            "xn_nm": pl.xn_nm[c],
            "table0": pl.t0,
            "xtg": pl.xtg[c],
            "wt": wt_np,
            "b_bc": bbc_np,
            "idx": pl.idx[c],
            "pcl": pcl,
        })
    res = run_pjrt_timed(nc, in_maps, reps=(10 if trace else 0))
    N = pl.N
    outf = np.zeros((N, D), dtype=np.float32)
    for c in range(NCORES):
        o = pl.core_nodes[c]
        rows = pl.p_of[o] * pl.G + pl.g_of[o]
        outf[o] = res.results[c]["out"][rows]
    return outf, res, pl


class TimedResults:
    pass


def run_pjrt_timed(nc, in_maps, reps=0):
    """Multi-core PJRT run mirroring bass2jax.run_bass_via_pjrt, but keeps the
    jitted executable + device inputs for repeated timed calls."""
    import time as _time
    import jax
    from jax.sharding import Mesh, PartitionSpec
    from jax.experimental.shard_map import shard_map
    from concourse import bass2jax, mybir
    from concourse.bass2jax import _bass_exec_p, partition_id_tensor, install_neuronx_cc_hook
    install_neuronx_cc_hook()
    n_cores = len(in_maps)
    partition_name = nc.partition_id_tensor.name if nc.partition_id_tensor else None
    in_names, out_names, out_avals, zero_outs = [], [], [], []
    for alloc in nc.m.functions[0].allocations:
        if not isinstance(alloc, mybir.MemoryLocationSet):
            continue
        name = alloc.memorylocations[0].name
        if alloc.kind == "ExternalInput":
            if name != partition_name:
                in_names.append(name)
        elif alloc.kind == "ExternalOutput":
            out_names.append(name)
            shape = tuple(alloc.tensor_shape)
            dtype = mybir.dt.np(alloc.dtype)
            out_avals.append(jax.core.ShapedArray(shape, dtype))
            zero_outs.append(np.zeros(shape, dtype))
    n_params = len(in_names)
    n_outs = len(out_avals)
    all_in_names = list(in_names) + list(out_names)
    if partition_name is not None:
        all_in_names.append(partition_name)

    def _body(*args):
        operands = list(args)
        if partition_name is not None:
            operands.append(partition_id_tensor())
        outs = _bass_exec_p.bind(
            *operands,
            out_avals=tuple(out_avals),
            in_names=tuple(all_in_names),
            out_names=tuple(out_names),
            lowering_input_output_aliases=(),
            sim_require_finite=True,
            sim_require_nnan=True,
            nc=nc,
        )
        return tuple(outs)

    devices = jax.devices()[:n_cores]
    mesh = Mesh(np.asarray(devices), ("core",))
    in_specs = (PartitionSpec("core"),) * (n_params + n_outs)
    out_specs = (PartitionSpec("core"),) * n_outs
    sharded = jax.jit(
        shard_map(_body, mesh=mesh, in_specs=in_specs, out_specs=out_specs,
                  check_rep=False),
        keep_unused=True,
    )
    concat_in = [
        np.concatenate([np.asarray(in_maps[c][nm]) for c in range(n_cores)], axis=0)
        for nm in in_names
    ]
    concat_zeros = [
        np.zeros((n_cores * z.shape[0], *z.shape[1:]), z.dtype) for z in zero_outs
    ]
    from jax.sharding import NamedSharding
    shardings = [NamedSharding(mesh, PartitionSpec("core"))] * (n_params + n_outs)
    dev_in = [jax.device_put(a, s) for a, s in zip(concat_in + concat_zeros, shardings)]
    out_arrs = sharded(*dev_in)
    jax.block_until_ready(out_arrs)
    times = []
    for _ in range(reps):
        t0 = _time.perf_counter()
        o = sharded(*dev_in)
        jax.block_until_ready(o)
        times.append(_time.perf_counter() - t0)
    res = TimedResults()
    res.results = [
        {name: np.asarray(out_arrs[i]).reshape(n_cores, *out_avals[i].shape)[c]
         for i, name in enumerate(out_names)}
        for c in range(n_cores)
    ]
    res.exec_time_ns = int(min(times) * 1e9) if times else None
    res.all_times_ms = [t * 1e3 for t in times]
    return res


def kernel(x, edge_index, W, b, betas):
    """Full-input entrypoint: shards, compiles (cached), runs on 8 cores,
    gathers the full output."""
    x = np.asarray(x)
    edge_index = np.asarray(edge_index)
    W = np.asarray(W)
    b = np.asarray(b)
    betas = np.asarray(betas)
    out, _res, _pl = run(x, edge_index, W, b, betas, B=8, CAP=24, GB=4, trace=False)
    return out.astype(np.float32)



# revision 2
# speedup vs baseline: 15.8040x; 15.8040x over previous
"""Self-contained AGNN kernel for trn2 x8 NeuronCores.

kernel(**inputs) takes the FULL unsharded inputs (x, edge_index, W, b, betas)
and returns the FULL [N, 64] float32 output. Internally: nodes are sharded
over 8 cores (snake by degree), edges grouped by destination into padded
[128-node x K] grids, per-edge source rows fetched with dma_gather (4 SWDGE
queues, one per source-window chunk) from a replicated bf16 feature table,
cosine-attention softmax-aggregation on the vector engine, and the table
rebuilt + all-gathered between layers. Layer-0 table is precomputed on host;
pad slots are corrected analytically via a per-node pad-count tile instead of
a bias column; the residual x@W.T+b runs up-front on the tensor engine and is
fused into the last layer's batch loop.
"""

import sys
sys.path.insert(0, '/opt/trn_rl_repo')
import numpy as np
import ml_dtypes
from concourse import bass, bacc, tile, library_config
import concourse.mybir as mybir
from concourse.bass_utils import run_bass_kernel_spmd

F32 = mybir.dt.float32
BF16 = mybir.dt.bfloat16
I16 = mybir.dt.int16
ALU = mybir.AluOpType
ACTF = mybir.ActivationFunctionType

P = 128          # partitions
D = 64           # feature dim
EW = 128         # table row elems (bf16) = 256B
NCORES = 8
NCHUNK = 4       # src windows (= core-pairs)


# ---------------------------------------------------------------- host side

class Plan:
    pass


def preprocess(x, edge_index, B=8, CAP=40):
    """Build per-core arrays + compile-time schedule.

    Node placement: rank nodes by degree, snake over cores (edge balance).
    Within a core, sort nodes by max-chunk-degree so groups of 128 have a
    tight shared K. Table row of node = core*SLOTS + p*G + g.
    """
    N = x.shape[0]
    assert N % NCORES == 0
    shard = N // NCORES
    G = (shard + P - 1) // P
    SLOTS = G * P
    NTAB = NCORES * SLOTS
    WIN = NTAB // NCHUNK           # rows per src window (= 2 cores)
    assert WIN <= 32767 and SLOTS * 2 == WIN

    # self-loops are NOT gathered: their softmax contribution (cos=1 with
    # itself) is added locally on-device as exp(beta) * h_v
    src = edge_index[0].astype(np.int64)
    dst = edge_index[1].astype(np.int64)
    deg = np.bincount(dst, minlength=N)

    # assign nodes to core-PAIRS (= gather source windows) with a greedy that
    # spreads each destination's in-edges evenly over the 4 windows: the
    # per-batch slot count K is max over windows of per-window in-degree, so
    # even spread cuts grid padding from ~1.58x toward ~ceil(deg/4)/deg*4
    so = np.argsort(src, kind="stable")
    ds_by_src = dst[so]
    cnt_out = np.bincount(src, minlength=N)
    indptr = np.r_[0, np.cumsum(cnt_out)]
    gorder = np.argsort(-cnt_out, kind="stable")
    cnt = np.zeros((NCHUNK, N), dtype=np.int32)
    sizes = np.zeros(NCHUNK, dtype=np.int64)
    capn = N // NCHUNK
    pair_of = np.empty(N, dtype=np.int64)
    for v in gorder:
        dv = ds_by_src[indptr[v]:indptr[v + 1]]
        if len(dv):
            sc = cnt[:, dv].sum(axis=1).astype(np.float64)
        else:
            sc = np.zeros(NCHUNK)
        sc[sizes >= capn] = np.inf
        p = int(np.argmin(sc))
        pair_of[v] = p
        sizes[p] += 1
        if len(dv):
            np.add.at(cnt, (p, dv), 1)
    # within each pair: snake the 2 cores by in-degree for dst-work balance
    core_of = np.empty(N, dtype=np.int64)
    for pr in range(NCHUNK):
        nodes = np.where(pair_of == pr)[0]
        o2 = nodes[np.argsort(-deg[nodes], kind="stable")]
        pos2 = np.arange(len(o2)) % 2
        blk2 = np.arange(len(o2)) // 2
        cr = np.where(blk2 % 2 == 0, pos2, 1 - pos2)
        core_of[o2] = pr * 2 + cr

    chunk = core_of[src] >> 1                       # edge chunk = src core-pair
    degc = np.zeros((NCHUNK, N), dtype=np.int64)    # per-chunk in-degree
    for c in range(NCHUNK):
        degc[c] = np.bincount(dst[chunk == c], minlength=N)
    maxstat = degc.max(axis=0)

    # within-core ordering: by maxstat desc (dummies land last)
    p_of = np.empty(N, dtype=np.int64)
    g_of = np.empty(N, dtype=np.int64)
    n_real = np.zeros(NCORES, dtype=np.int64)
    kmax_pg = np.zeros((NCORES, G), dtype=np.int64)  # per (core, group) max stat
    core_nodes = []
    for c in range(NCORES):
        nodes = np.where(core_of == c)[0]
        n_real[c] = len(nodes)
        assert len(nodes) < SLOTS, "need >=1 dummy slot per core"
        o = nodes[np.argsort(-maxstat[nodes], kind="stable")]
        s = np.arange(len(o))
        g_of[o] = s // P
        p_of[o] = s % P
        core_nodes.append(o)
        gm = np.zeros(G, dtype=np.int64)
        np.maximum.at(gm, s // P, maxstat[o])
        kmax_pg[c] = gm

    row_of = core_of * SLOTS + p_of * G + g_of       # table row per real node

    # batch schedule (shared across cores): groups [g0, g0+Bb), K = max over
    # cores; pack groups while Bb*K stays under CAP (bounds tile sizes)
    def kfor(g0, g1):
        K = int(kmax_pg[:, g0:g1].max())
        return max(2, K)
    batches = []
    g0 = 0
    while g0 < G:
        Bb = 1
        while (g0 + Bb < G and Bb < B
               and (Bb + 1) * kfor(g0, g0 + Bb + 1) <= CAP):
            Bb += 1
        batches.append((g0, Bb, kfor(g0, g0 + Bb)))
        g0 += Bb

    # per-(batch,chunk) idx columns layout in the "idx" param
    col_off = []
    tot = 0
    for (g0, Bb, K) in batches:
        offs = []
        for c in range(NCHUNK):
            offs.append(tot)
            tot += 8 * Bb * K                        # cols = num_idxs/16
        col_off.append(offs)
    S_TOTAL = tot

    # dummy (pad) relative row per window: first dummy slot of core 2c
    pad_rel = np.empty(NCHUNK, dtype=np.int64)
    for c in range(NCHUNK):
        cc = 2 * c
        s = n_real[cc]                               # first dummy rank
        pad_rel[c] = (s % P) * G + s // P

    # slot k of each edge within its (dst, chunk) run
    key = dst * NCHUNK + chunk
    o = np.argsort(key, kind="stable")
    ks = np.empty(len(o), dtype=np.int64)
    sk = key[o]
    run_start = np.r_[0, np.flatnonzero(np.diff(sk)) + 1]
    run_id = np.zeros(len(o), dtype=np.int64)
    run_id[run_start[1:]] = 1
    run_id = np.cumsum(run_id)
    ks[o] = np.arange(len(o)) - run_start[run_id]

    # batch id / K per group
    batch_of_g = np.empty(G, dtype=np.int64)
    K_of_g = np.empty(G, dtype=np.int64)
    for bi, (g0, Bb, K) in enumerate(batches):
        batch_of_g[g0:g0 + Bb] = bi
        K_of_g[g0:g0 + Bb] = K

    # flat idx arrays per core, filled with per-window pad rows
    idx_arr = np.empty((NCORES, S_TOTAL * 16), dtype=np.int16)
    for bi, (g0, Bb, K) in enumerate(batches):
        for c in range(NCHUNK):
            lo = col_off[bi][c] * 16
            idx_arr[:, lo:lo + 128 * Bb * K] = pad_rel[c]

    # scatter real edges
    ec = core_of[dst]
    eg = g_of[dst]
    ep = p_of[dst]
    ebi = batch_of_g[eg]
    eK = K_of_g[eg]
    g0_of_b = np.array([b[0] for b in batches], dtype=np.int64)
    off_bc = np.array(col_off, dtype=np.int64)       # [nb, 4]
    base = off_bc[ebi, chunk] * 16
    pos = base + (((eg - g0_of_b[ebi]) * eK + ks) * P + ep)
    val = (row_of[src] - chunk * WIN).astype(np.int16)
    assert (ks < eK).all()
    idx_arr[ec, pos] = val

    # -> [core, 128, S_TOTAL] wrapped 16 + replicated
    idx_parts = np.empty((NCORES, P, S_TOTAL), dtype=np.int16)
    for cc in range(NCORES):
        w = idx_arr[cc].reshape(S_TOTAL, 16).T       # [16, S]
        idx_parts[cc] = np.tile(w, (8, 1))

    # per-core node-ordered inputs
    x_nm = np.zeros((NCORES, P, G * D), dtype=np.float32)
    xn_nm = np.zeros((NCORES, P, G * D), dtype=ml_dtypes.bfloat16)
    xtg = np.zeros((NCORES, D, G * P), dtype=ml_dtypes.bfloat16)
    t0 = np.zeros((NTAB, EW), dtype=ml_dtypes.bfloat16)   # replicated layer-0 table
    degg = np.zeros((NCORES, P, G), dtype=np.float32)     # total degree per slot

    xf = x.astype(np.float32)
    norm = np.maximum(np.sqrt((xf * xf).sum(1)), 1e-12)
    xn_full = (xf / norm[:, None])

    for cc in range(NCORES):
        o = core_nodes[cc]
        pp, gg = p_of[o], g_of[o]
        xo = xf[o]
        flat = np.zeros((P, G, D), dtype=np.float32)
        flat[pp, gg] = xo
        x_nm[cc] = flat.reshape(P, G * D)
        flatn = np.zeros((P, G, D), dtype=np.float32)
        flatn[pp, gg] = xn_full[o]
        xn_nm[cc] = flatn.reshape(P, G * D).astype(ml_dtypes.bfloat16)
        xt = np.zeros((G, P, D), dtype=np.float32)
        xt[gg, pp] = xo
        xtg[cc] = xt.reshape(G * P, D).T.astype(ml_dtypes.bfloat16)
        dg = np.zeros((P, G), dtype=np.float32)
        dg[pp, gg] = deg[o]
        degg[cc] = dg
        # layer-0 table rows for this core (row = p*G+g): xn + norm col
        trows = np.zeros((P, G, EW), dtype=np.float32)
        trows[pp, gg, 0:D] = xn_full[o]
        nrm = np.full((P, G), 1e-12, dtype=np.float32)
        nrm[pp, gg] = norm[o]
        trows[:, :, D] = nrm
        t0[cc * SLOTS:(cc + 1) * SLOTS] = trows.reshape(SLOTS, EW).astype(ml_dtypes.bfloat16)

    # pad-count correction per (p, g): NCHUNK*K_b(g) - deg  (subtract from
    # the raw denominator; e^beta self-loop term added in run())
    padc = np.zeros((NCORES, P, G), dtype=np.float32)
    for cc in range(NCORES):
        padc[cc] = NCHUNK * K_of_g[None, :].astype(np.float32) - degg[cc]

    pl = Plan()
    pl.N, pl.G, pl.SLOTS, pl.NTAB, pl.WIN, pl.S_TOTAL = N, G, SLOTS, NTAB, WIN, S_TOTAL
    pl.batches, pl.col_off = batches, col_off
    pl.idx = idx_parts
    pl.x_nm, pl.xn_nm, pl.xtg, pl.t0 = x_nm, xn_nm, xtg, t0
    pl.padc = padc
    pl.row_of = row_of
    pl.core_nodes = core_nodes
    pl.p_of, pl.g_of = p_of, g_of
    padded = sum(128 * Bb * K * NCHUNK for (g0, Bb, K) in batches) * NCORES
    pl.pad_factor = padded / len(src)
    return pl


# ---------------------------------------------------------------- device side

def build_kernel(pl, betas, debug=False, GB=4, DOTRED=True, NUMRED=False):
    G, WIN, NTAB, SLOTS = pl.G, pl.WIN, pl.NTAB, pl.SLOTS
    L = len(betas)
    nc = bacc.Bacc("TRN2", target_bir_lowering=False, debug=debug,
                   num_devices=NCORES, num_swdge_queues=4)

    x_nm = nc.dram_tensor("x_nm", [P, G * D], F32, kind="ExternalInput")
    xn_nm = nc.dram_tensor("xn_nm", [P, G * D], BF16, kind="ExternalInput")
    table0 = nc.dram_tensor("table0", [NTAB, EW], BF16, kind="ExternalInput")
    xtg = nc.dram_tensor("xtg", [D, G * P], BF16, kind="ExternalInput")
    wt = nc.dram_tensor("wt", [D, D], BF16, kind="ExternalInput")
    b_bc = nc.dram_tensor("b_bc", [P, D], F32, kind="ExternalInput")
    idx = nc.dram_tensor("idx", [P, pl.S_TOTAL], I16, kind="ExternalInput")
    pcl = nc.dram_tensor("pcl", [P, L * G], F32, kind="ExternalInput")
    out = nc.dram_tensor("out", [SLOTS, D], F32, kind="ExternalOutput")

    with tile.TileContext(nc) as tc:
        nc.gpsimd.load_library(library_config.mlp)
        with tc.tile_pool(name="persist", bufs=1) as wp, \
             tc.tile_pool(name="gat", bufs=GB) as gp, \
             tc.tile_pool(name="idxp", bufs=2) as ip, \
             tc.tile_pool(name="scr", bufs=1) as sp, \
             tc.tile_pool(name="town", bufs=2) as tp, \
             tc.tile_pool(name="fin", bufs=2) as fp_, \
             tc.tile_pool(name="psum", bufs=2, space="PSUM") as pp, \
             tc.tile_pool(name="dram", bufs=1, space="DRAM") as dp:

            H = wp.tile([P, G * D], F32)
            XN = wp.tile([P, G * D], BF16)
            INV = wp.tile([P, G], F32)
            NORMB = wp.tile([P, G], BF16)
            PC = wp.tile([P, L * G], F32)
            t_own = dp.tile([SLOTS, EW], BF16)
            rdram = dp.tile([P, G * D], F32)
            tables = [dp.tile([NTAB, EW], BF16, addr_space="Shared",
                              name=f"table{i}", tag=f"table{i}")
                      for i in range(1, L)]

            nc.sync.dma_start(out=H[:], in_=x_nm.ap())
            nc.sync.dma_start(out=XN[:], in_=xn_nm.ap())
            nc.sync.dma_start(out=PC[:], in_=pcl.ap())

            def h3():
                return H[:].rearrange("p (g e) -> p g e", g=G)

            def xn3():
                return XN[:].rearrange("p (g e) -> p g e", g=G)

            # ---- residual x @ W.T + b, computed up front into rdram
            WT = wp.tile([D, D], BF16)
            nc.sync.dma_start(out=WT[:], in_=wt.ap())
            BBC = wp.tile([P, D], F32)
            nc.sync.dma_start(out=BBC[:], in_=b_bc.ap())
            SLAB = 8
            for s0 in range(0, G, SLAB):
                sb = min(SLAB, G - s0)
                XT = tp.tile([D, SLAB * P], BF16, tag="xt")
                nc.sync.dma_start(out=XT[:, :sb * P], in_=xtg.ap()[:, s0 * P:(s0 + sb) * P])
                PS = pp.tile([P, SLAB * D], F32, space="PSUM", tag="ps")
                for j in range(sb):
                    nc.tensor.matmul(
                        out=PS[:, j * D:(j + 1) * D],
                        lhsT=XT[:, j * P:(j + 1) * P], rhs=WT[:],
                        start=True, stop=True)
                RS = tp.tile([P, SLAB * D], F32, tag="rs")
                bb = BBC[:].unsqueeze(1).broadcast_to([P, sb, D])
                nc.vector.tensor_tensor(
                    out=RS[:, :sb * D].rearrange("p (g e) -> p g e", g=sb),
                    in0=PS[:, :sb * D].rearrange("p (g e) -> p g e", g=sb),
                    in1=bb, op=ALU.add)
                nc.sync.dma_start(out=rdram[:, s0 * D:(s0 + sb) * D], in_=RS[:, :sb * D])

            def build_table(li):
                # norms of H rows -> INV, XN (normalized, bf16); write xn+norm
                # columns of t_own via strided DMAs; allgather -> table
                table = tables[li - 1]
                SQ = sp.tile([P, G * D], F32, tag="pr")
                nc.vector.tensor_tensor(out=SQ[:], in0=H[:], in1=H[:], op=ALU.mult)
                NSQ = sp.tile([P, G], F32, tag="nsq")
                nc.vector.tensor_reduce(
                    out=NSQ[:], in_=SQ[:].rearrange("p (g e) -> p g e", g=G),
                    axis=mybir.AxisListType.X, op=ALU.add)
                NRM = sp.tile([P, G], F32, tag="nrm")
                nc.scalar.activation(out=NRM[:], in_=NSQ[:], func=ACTF.Sqrt)
                nc.vector.tensor_scalar_max(out=NRM[:], in0=NRM[:], scalar1=1e-12)
                nc.vector.reciprocal(out=INV[:], in_=NRM[:])
                nc.vector.tensor_copy(out=NORMB[:], in_=NRM[:])
                inv3 = INV[:].unsqueeze(2).broadcast_to([P, G, D])
                nc.vector.tensor_tensor(out=xn3(), in0=h3(), in1=inv3, op=ALU.mult)
                tv = t_own[:].rearrange("(p g) e -> p g e", p=P)
                SLAB = 32   # keep DMA descriptor count per instruction < 8192
                for s0 in range(0, G, SLAB):
                    sb = min(SLAB, G - s0)
                    nc.sync.dma_start(
                        out=tv[:, s0:s0 + sb, 0:D],
                        in_=xn3()[:, s0:s0 + sb, :])
                    nc.sync.dma_start(
                        out=tv[:, s0:s0 + sb, D:D + 1],
                        in_=NORMB[:, s0:s0 + sb].unsqueeze(2))
                nc.gpsimd.collective_compute(
                    "AllGather", ALU.bypass,
                    replica_groups=[list(range(NCORES))],
                    ins=[t_own[:].opt()],
                    outs=[table[:].opt()])

            for li in range(L):
                beta = float(betas[li])
                ebeta = float(np.exp(beta))
                table_ap = table0.ap() if li == 0 else tables[li - 1][:]
                last = li == L - 1
                for bi, (g0, Bb, K) in enumerate(pl.batches):
                    W4 = NCHUNK * Bb * K              # slots per partition
                    cols = 8 * Bb * K                 # idx cols per chunk
                    IX = ip.tile([P, NCHUNK * cols], I16, tag="ix")
                    nc.sync.dma_start(
                        out=IX[:],
                        in_=idx.ap()[:, pl.col_off[bi][0]:pl.col_off[bi][0] + NCHUNK * cols])
                    GT = gp.tile([P, W4 * EW], BF16, tag="gt")
                    for c in range(NCHUNK):
                        slab = GT[:, c * Bb * K * EW:(c + 1) * Bb * K * EW]
                        nc.gpsimd.dma_gather(
                            slab.rearrange("p (j e) -> p j e", e=EW),
                            table_ap[c * WIN:(c + 1) * WIN, :],
                            IX[:, c * cols:(c + 1) * cols],
                            128 * Bb * K, 128 * Bb * K, EW, elem_step=EW,
                            single_packet=False, queue_num=c)
                    gt4 = GT[:].rearrange("p (j e) -> p j e", e=EW)

                    # dot products: per chunk multiply with xn broadcast
                    PR = sp.tile([P, W4 * D], BF16, tag="pr")
                    pr4c = PR[:].rearrange("p (cb k e) -> p cb k e", k=K, e=D)
                    xb = (xn3()[:, g0:g0 + Bb, :].unsqueeze(2)
                          .broadcast_to([P, Bb, K, D]))
                    for c in range(NCHUNK):
                        nc.vector.tensor_tensor(
                            out=pr4c[:, c * Bb:(c + 1) * Bb, :, :],
                            in0=gt4[:, c * Bb * K:(c + 1) * Bb * K, 0:D]
                                .rearrange("p (b k) e -> p b k e", k=K),
                            in1=xb, op=ALU.mult)

                    DOT = sp.tile([P, W4], F32, tag="dot")
                    if DOTRED:
                        nc.vector.tensor_reduce(
                            out=DOT[:],
                            in_=PR[:].rearrange("p (j e) -> p j e", e=D),
                            axis=mybir.AxisListType.X, op=ALU.add)
                    else:
                        TR = sp.tile([P, W4 * 32], BF16, tag="tr")
                        tr4 = TR[:].rearrange("p (j e) -> p j e", e=32)
                        prj = PR[:].rearrange("p (j e) -> p j e", e=D)
                        nc.vector.tensor_tensor(out=tr4[:, :, 0:32], in0=prj[:, :, 0:32],
                                                in1=prj[:, :, 32:64], op=ALU.add)
                        w = 16
                        while w >= 2:
                            nc.vector.tensor_tensor(
                                out=tr4[:, :, 0:w], in0=tr4[:, :, 0:w],
                                in1=tr4[:, :, w:2 * w], op=ALU.add)
                            w //= 2
                        nc.vector.tensor_reduce(out=DOT[:], in_=tr4[:, :, 0:2],
                                                axis=mybir.AxisListType.X, op=ALU.add)

                    EX = sp.tile([P, W4], BF16, tag="ex")
                    nc.scalar.activation(out=EX[:], in_=DOT[:],
                                         func=ACTF.Exp, scale=beta)
                    # denominator: sum + pad-count/self-loop correction
                    DEN = sp.tile([P, Bb], F32, tag="den")
                    exv = EX[:].rearrange("p (c b k) -> p b c k", c=NCHUNK, k=K)
                    nc.vector.tensor_reduce(
                        out=DEN[:], in_=exv,
                        axis=mybir.AxisListType.XY, op=ALU.add)
                    nc.vector.tensor_tensor(
                        out=DEN[:], in0=DEN[:],
                        in1=PC[:, li * G + g0:li * G + g0 + Bb], op=ALU.subtract)
                    R = sp.tile([P, Bb], F32, tag="rcp")
                    nc.vector.reciprocal(out=R[:], in_=DEN[:])
                    # weight including src norm (strided col D of the gather)
                    EXN = sp.tile([P, W4], BF16, tag="exn")
                    nc.vector.tensor_tensor(out=EXN[:], in0=EX[:],
                                            in1=gt4[:, :, D], op=ALU.mult)
                    # numer: PR reuse = GT * exn-broadcast, reduce over (c,k)
                    exb = EXN[:].unsqueeze(2).broadcast_to([P, W4, D])
                    pr4 = PR[:].rearrange("p (cb k e) -> p cb k e", k=K, e=D)
                    nc.vector.tensor_tensor(
                        out=pr4.rearrange("p cb k e -> p (cb k) e"),
                        in0=gt4[:, :, 0:D], in1=exb, op=ALU.mult)
                    NUM = sp.tile([P, Bb * D], F32, tag="num")
                    n3 = NUM[:].rearrange("p (b e) -> p b e", b=Bb)
                    if NUMRED:
                        nc.vector.tensor_reduce(
                            out=n3,
                            in_=PR[:].rearrange("p (c b k e) -> p b e c k",
                                                c=NCHUNK, k=K, e=D),
                            axis=mybir.AxisListType.XY, op=ALU.add)
                    else:
                        w = K
                        while w > 1:
                            if w % 2 == 1:
                                nc.vector.tensor_tensor(
                                    out=pr4[:, :, 0, :], in0=pr4[:, :, 0, :],
                                    in1=pr4[:, :, w - 1, :], op=ALU.add)
                                w -= 1
                            else:
                                nc.vector.tensor_tensor(
                                    out=pr4[:, :, 0:w // 2, :], in0=pr4[:, :, 0:w // 2, :],
                                    in1=pr4[:, :, w // 2:w, :], op=ALU.add)
                                w //= 2
                        nc.vector.tensor_tensor(out=n3, in0=pr4[:, 0:Bb, 0, :],
                                                in1=pr4[:, Bb:2 * Bb, 0, :], op=ALU.add)
                        nc.vector.tensor_tensor(out=n3, in0=n3,
                                                in1=pr4[:, 2 * Bb:3 * Bb, 0, :], op=ALU.add)
                        nc.vector.tensor_tensor(out=n3, in0=n3,
                                                in1=pr4[:, 3 * Bb:4 * Bb, 0, :], op=ALU.add)
                    # self-loop numerator term + normalize
                    nc.vector.scalar_tensor_tensor(
                        out=n3, in0=h3()[:, g0:g0 + Bb, :],
                        scalar=ebeta, in1=n3,
                        op0=ALU.mult, op1=ALU.add)
                    rb = R[:].unsqueeze(2).broadcast_to([P, Bb, D])
                    if not last:
                        nc.vector.tensor_tensor(
                            out=h3()[:, g0:g0 + Bb, :], in0=n3, in1=rb, op=ALU.mult)
                    else:
                        # fuse: out = relu(h_conv + res)
                        RSL = fp_.tile([P, CAPB * D], F32, tag="rsl")
                        nc.sync.dma_start(out=RSL[:, :Bb * D],
                                          in_=rdram[:, g0 * D:(g0 + Bb) * D])
                        FIN = fp_.tile([P, CAPB * D], F32, tag="fin")
                        f3 = FIN[:, :Bb * D].rearrange("p (b e) -> p b e", b=Bb)
                        nc.vector.tensor_tensor(out=f3, in0=n3, in1=rb, op=ALU.mult)
                        nc.vector.tensor_tensor(
                            out=f3, in0=f3,
                            in1=RSL[:, :Bb * D].rearrange("p (b e) -> p b e", b=Bb),
                            op=ALU.add)
                        nc.scalar.activation(out=FIN[:, :Bb * D], in_=FIN[:, :Bb * D],
                                             func=ACTF.Relu)
                        nc.sync.dma_start(
                            out=out.ap().rearrange("(p g) e -> p (g e)", p=P)
                            [:, g0 * D:(g0 + Bb) * D],
                            in_=FIN[:, :Bb * D])
                if li == 0:
                    nc.scalar.activation(out=H[:], in_=H[:], func=ACTF.Relu)
                if li < L - 1:
                    build_table(li + 1)
    nc.compile()
    return nc


CAPB = 8  # max Bb for the fin tile size


# ---------------------------------------------------------------- runner

_CACHE = {}


def run(x, edge_index, W, b, betas, B=8, CAP=24, GB=4, DOTRED=True, NUMRED=False,
        trace=False):
    global CAPB
    CAPB = B
    key = (x.shape, edge_index.shape, B, CAP, GB, DOTRED, NUMRED)
    pl = preprocess(np.asarray(x), np.asarray(edge_index), B=B, CAP=CAP)
    if key in _CACHE:
        nc = _CACHE[key]
    else:
        nc = build_kernel(pl, np.asarray(betas), GB=GB, DOTRED=DOTRED, NUMRED=NUMRED)
        _CACHE[key] = nc

    betas_np = np.asarray(betas, dtype=np.float32)
    L = len(betas_np)
    G = pl.G
    wt_np = np.ascontiguousarray(np.asarray(W).T).astype(ml_dtypes.bfloat16)
    bbc_np = np.tile(np.asarray(b, dtype=np.float32)[None, :], (P, 1))
    in_maps = []
    for c in range(NCORES):
        # pcl[l] = padcount - e^beta_l  (device does DEN = sum - pcl)
        pcl = np.empty((P, L * G), dtype=np.float32)
        for l in range(L):
            pcl[:, l * G:(l + 1) * G] = pl.padc[c] - np.exp(betas_np[l])
        in_maps.append({
            "x_nm": pl.x_nm[c],
            "xn_nm": pl.xn_nm[c],
            "table0": pl.t0,
            "xtg": pl.xtg[c],
            "wt": wt_np,
            "b_bc": bbc_np,
            "idx": pl.idx[c],
            "pcl": pcl,
        })
    res = run_pjrt_timed(nc, in_maps, reps=(10 if trace else 0))
    N = pl.N
    outf = np.zeros((N, D), dtype=np.float32)
    for c in range(NCORES):
        o = pl.core_nodes[c]
        rows = pl.p_of[o] * pl.G + pl.g_of[o]
        outf[o] = res.results[c]["out"][rows]
    return outf, res, pl


class TimedResults:
    pass


def run_pjrt_timed(nc, in_maps, reps=0):
    """Multi-core PJRT run mirroring bass2jax.run_bass_via_pjrt, but keeps the
    jitted executable + device inputs for repeated timed calls."""
    import time as _time
    import jax
    from jax.sharding import Mesh, PartitionSpec
    from jax.experimental.shard_map import shard_map
    from concourse import bass2jax, mybir
    from concourse.bass2jax import _bass_exec_p, partition_id_tensor, install_neuronx_cc_hook
    install_neuronx_cc_hook()
    n_cores = len(in_maps)
    partition_name = nc.partition_id_tensor.name if nc.partition_id_tensor else None
    in_names, out_names, out_avals, zero_outs = [], [], [], []
    for alloc in nc.m.functions[0].allocations:
        if not isinstance(alloc, mybir.MemoryLocationSet):
            continue
        name = alloc.memorylocations[0].name
        if alloc.kind == "ExternalInput":
            if name != partition_name:
                in_names.append(name)
        elif alloc.kind == "ExternalOutput":
            out_names.append(name)
            shape = tuple(alloc.tensor_shape)
            dtype = mybir.dt.np(alloc.dtype)
            out_avals.append(jax.core.ShapedArray(shape, dtype))
            zero_outs.append(np.zeros(shape, dtype))
    n_params = len(in_names)
    n_outs = len(out_avals)
    all_in_names = list(in_names) + list(out_names)
    if partition_name is not None:
        all_in_names.append(partition_name)

    def _body(*args):
        operands = list(args)
        if partition_name is not None:
            operands.append(partition_id_tensor())
        outs = _bass_exec_p.bind(
            *operands,
            out_avals=tuple(out_avals),
            in_names=tuple(all_in_names),
            out_names=tuple(out_names),
            lowering_input_output_aliases=(),
            sim_require_finite=True,
            sim_require_nnan=True,
            nc=nc,
        )
        return tuple(outs)

    devices = jax.devices()[:n_cores]
    mesh = Mesh(np.asarray(devices), ("core",))
    in_specs = (PartitionSpec("core"),) * (n_params + n_outs)
    out_specs = (PartitionSpec("core"),) * n_outs
    sharded = jax.jit(
        shard_map(_body, mesh=mesh, in_specs=in_specs, out_specs=out_specs,
                  check_rep=False),
        keep_unused=True,
    )
    concat_in = [
        np.concatenate([np.asarray(in_maps[c][nm]) for c in range(n_cores)], axis=0)
        for nm in in_names
    ]
    concat_zeros = [
        np.zeros((n_cores * z.shape[0], *z.shape[1:]), z.dtype) for z in zero_outs
    ]
    from jax.sharding import NamedSharding
    shardings = [NamedSharding(mesh, PartitionSpec("core"))] * (n_params + n_outs)
    dev_in = [jax.device_put(a, s) for a, s in zip(concat_in + concat_zeros, shardings)]
    out_arrs = sharded(*dev_in)
    jax.block_until_ready(out_arrs)
    times = []
    for _ in range(reps):
        t0 = _time.perf_counter()
        o = sharded(*dev_in)
        jax.block_until_ready(o)
        times.append(_time.perf_counter() - t0)
    res = TimedResults()
    res.sharded = sharded
    res.dev_in = dev_in
    res.results = [
        {name: np.asarray(out_arrs[i]).reshape(n_cores, *out_avals[i].shape)[c]
         for i, name in enumerate(out_names)}
        for c in range(n_cores)
    ]
    res.exec_time_ns = int(min(times) * 1e9) if times else None
    res.all_times_ms = [t * 1e3 for t in times]
    return res


def kernel(x, edge_index, W, b, betas):
    """Full-input entrypoint: shards, compiles (cached), runs on 8 cores,
    gathers the full output."""
    x = np.asarray(x)
    edge_index = np.asarray(edge_index)
    W = np.asarray(W)
    b = np.asarray(b)
    betas = np.asarray(betas)
    out, _res, _pl = run(x, edge_index, W, b, betas, B=8, CAP=24, GB=4, trace=False)
    return out.astype(np.float32)

